# revision 16
# baseline (speedup 1.0000x reference)
"""Trainium2 Bass kernel for ASN consistency loss.

Math: the 1x1-conv heads have C_in=1, so relu(x*W1[c]) splits into
alpha/beta coefficients applied to relu(x)/relu(-x).  The per-window
cosine distances then reduce to windowed sums of 8 pointwise quantities
{x^2, y^2, xp^2, yp^2, xp*yp, xp*y, x*yp, x*y} with xp=relu(x), combined
with a handful of scalars derived from the weights.  The windowed sums
are one accumulated float32r matmul against a 0/1 window-indicator
matrix with time on the partition (contraction) axis.  The final loss
only needs per-sample sums of cos-sim and relu(cos-sim - (1-margin))
over all (mel, window) pairs, so each core emits a [128, 128] block of
per-window values; the cheap reductions finish on host.

Sharding: data-parallel over batch, 4 samples/core on 8 cores.  The 16
selected mel rows are gathered and the tile is laid out time-major on
host (pure data marshalling; all FLOPs stay on device).
"""

import numpy as np

import concourse.bass as bass
import concourse.tile as tile
from concourse import bacc, mybir
from concourse.bass import ts
from concourse.bass_utils import run_bass_kernel_spmd

# ---- constants from the nn.Module (hardcoded) ----
SR = 16000
HOP_LENGTH = 256
HOP_MS = 40
WIN_LENGTHS = (80, 160)
MARGIN = 0.2
SPOOF_WEIGHT = 0.5
MEL_SAMPLE = 16
MAX_TIME_WINDOWS = 64
SEED = 123

B, M, T = 32, 80, 2000
N_CORES = 8
B_LOC = B // N_CORES        # 4 samples per core
NMEL = MEL_SAMPLE           # 16
BM = B_LOC * NMEL           # 64 (b, mel) pairs per core
TPAD = 2048
NCHUNK = 16                 # t chunks of 128
NW = 128                    # 64 windows of w=5 + 64 of w=10
NQ = 8                      # pointwise quantities
NWIN_TOT = 2 * MAX_TIME_WINDOWS * NMEL  # 2048 window-mel pairs per sample
F32 = mybir.dt.float32
F32R = mybir.dt.float32r

NEWTON_ITERS = 1  # rsqrt refinement steps after ACT sqrt + DVE reciprocal


def _ms_to_frames(ms):
    samples = int(SR * (ms / 1000.0))
    return max(3, samples // HOP_LENGTH)


def _window_meta():
    hop = _ms_to_frames(HOP_MS)
    out = []
    for w_ms in WIN_LENGTHS:
        w = _ms_to_frames(w_ms)
        starts = np.arange(0, T - w + 1, hop, dtype=np.int64)
        if MAX_TIME_WINDOWS and starts.size > MAX_TIME_WINDOWS:
            sel = np.linspace(0, starts.size - 1, MAX_TIME_WINDOWS).astype(np.int64)
            starts = starts[sel]
        out.append((starts, w))
    return out


def _build_s():
    """Window indicator matrix, SBUF layout [128 part, NCHUNK, NW]:
    s[p, c, win] = 1.0 iff t = c*128 + p lies inside window `win`."""
    S = np.zeros((TPAD, NW), np.float32)
    col = 0
    for starts, w in _window_meta():
        for s0 in starts:
            S[s0:s0 + w, col] = 1.0
            col += 1
    assert col == NW
    return np.ascontiguousarray(S.reshape(NCHUNK, 128, NW).transpose(1, 0, 2))


IDX_M = np.sort(np.random.default_rng(SEED).permutation(M)[:MEL_SAMPLE])


def _derived_scalars(W1a, W2a, W1p, W2p):
    def alphabeta(W1, W2):
        w = W1[:, 0].astype(np.float64)
        W2 = W2.astype(np.float64)
        alpha = (W2 * (w * (w > 0))[None, :]).sum(1)
        beta = (W2 * ((-w) * (w < 0))[None, :]).sum(1)
        return alpha, beta

    aa, ba = alphabeta(W1a, W2a)
    ap, bp = alphabeta(W1p, W2p)
    caa = aa @ ap; cab = aa @ bp; cba = ba @ ap; cbb = ba @ bp
    Aa = aa @ aa; Ba = ba @ ba; Ap_ = ap @ ap; Bp_ = bp @ bp
    # q = k1 xp*yp + k2 xp*y + k3 x*yp + k4 x*y  (scaled by 1/sqrt(Ba*Bp))
    kscale = 1.0 / np.sqrt(Ba * Bp_)
    k1 = (caa + cab + cba + cbb) * kscale
    k2 = -(cab + cbb) * kscale
    k3 = -(cba + cbb) * kscale
    k4 = cbb * kscale
    # gu = Ba*(x^2 + r1*xp^2), gv = Bp*(y^2 + r2*yp^2)
    r1 = (Aa - Ba) / Ba
    r2 = (Ap_ - Bp_) / Bp_
    rA = r1 if r1 > 0 else 1.0
    return tuple(
        float(v)
        for v in (
            k1, k2, k3, k4,
            np.sqrt(rA),        # scale inside Square -> slot2 = rA*xp^2
            r1 / rA,            # gu = P0 + guc*P2   (1.0 when folded)
            r2 / rA,            # gv = P1 + gvc*P3
        )
    )


def _build_bass(consts):
    k1, k2, k3, k4, s_sq, guc, gvc = consts
    Relu = mybir.ActivationFunctionType.Relu
    Sq = mybir.ActivationFunctionType.Square
    AOP = mybir.AluOpType

    nc = bacc.Bacc()
    xy_in = nc.dram_tensor("xy", [128, NCHUNK, 128], F32, kind="ExternalInput")
    res_out = nc.dram_tensor("res", [128, 2 * BM], F32, kind="ExternalOutput")
    s_dram = nc.inline_tensor(_build_s(), name="smat")

    with tile.TileContext(nc) as tc:
        with (
            tc.tile_pool(name="main", bufs=1) as pool,
            tc.tile_pool(name="acc", bufs=1, space="PSUM") as psum_acc,
        ):
            # Pin the ACT table set (sqrt_and_others covers Sqrt/Relu/Square)
            # with a tiny early sqrt so no mid-kernel table switch happens.
            scr = pool.tile([1, 1], F32)
            nc.scalar.sqrt(scr[:], nc.const_aps.tensor(1.0, (1, 1)))

            margin_bias = pool.tile([128, 1], F32)
            nc.gpsimd.memset(margin_bias[:], -(1.0 - MARGIN))

            # input is pre-transposed on host: [t%128, chunk, bmel-xy]
            xyT = pool.tile([128, NCHUNK, 128], F32)
            nc.sync.dma_start(xyT[:, 0:NCHUNK // 2, :], xy_in[:, 0:NCHUNK // 2, :])
            nc.scalar.dma_start(xyT[:, NCHUNK // 2:, :], xy_in[:, NCHUNK // 2:, :])

            s_sb = pool.tile([128, NCHUNK, NW], F32R)
            nc.gpsimd.dma_start(s_sb[:].bitcast(F32), s_dram[:])

            # ---- pointwise quantities ----
            XPYP = pool.tile([128, NCHUNK, 128], F32)
            Q = pool.tile([128, NCHUNK, NQ * BM], F32R)

            # relu in quarters on gpsimd so products can start early
            for g in range(4):
                nc.gpsimd.tensor_scalar(
                    XPYP[:, ts(g, 4), :], xyT[:, ts(g, 4), :], 0.0, None, AOP.max
                )

            halves = [slice(0, NCHUNK // 2), slice(NCHUNK // 2, NCHUNK)]
            for h in halves:
                XY = xyT[:, h, :]
                X = xyT[:, h, 0:BM]
                Y = xyT[:, h, BM:128]
                XPh = XPYP[:, h, 0:BM]
                YPh = XPYP[:, h, BM:128]

                def qs(i, h=h):
                    return Q[:, h, ts(i, BM)]

                nc.vector.tensor_mul(out=qs(7), in0=X, in1=Y)          # x*y
                nc.scalar.activation(Q[:, h, 0:128], XY, Sq)           # x^2|y^2
                nc.vector.tensor_mul(out=qs(6), in0=X, in1=YPh)        # x*yp
                nc.scalar.activation(Q[:, h, 128:256], XPYP[:, h, :], Sq,
                                     scale=s_sq)                       # rA*(xp^2|yp^2)
                nc.vector.tensor_mul(out=qs(5), in0=XPh, in1=Y)        # xp*y
                nc.vector.tensor_mul(out=qs(4), in0=XPh, in1=YPh)      # xp*yp

            # ---- windowed sums: accumulate over the 16 t-chunks ----
            Pacc = psum_acc.tile([128, NQ * BM], F32, tag="pacc")
            for c in range(NCHUNK):
                nc.tensor.matmul(
                    Pacc[:],
                    s_sb[:, c, :],
                    Q[:, c, :],
                    start=(c == 0),
                    stop=(c == NCHUNK - 1),
                )

            def Pi(i):
                return Pacc[:, ts(i, BM)]

            # ---- cosine + margin terms, [128 windows, 64 bmel] ----
            # (each op touches at most one PSUM operand)
            e1 = pool.tile([128, BM], F32)
            e2 = pool.tile([128, BM], F32)
            den = pool.tile([128, BM], F32)
            rs = pool.tile([128, BM], F32)
            nt = pool.tile([128, BM], F32)
            qn = pool.tile([128, BM], F32)
            qa = pool.tile([128, BM], F32)
            qb = pool.tile([128, BM], F32)
            qc = pool.tile([128, BM], F32)
            CS = pool.tile([128, 2 * BM], F32)

            # gu = x2 + guc*xp2' ; gv = y2 + gvc*yp2'
            nc.scalar.mul(e1[:], Pi(2), guc)
            nc.vector.tensor_add(out=e1[:], in0=e1[:], in1=Pi(0))
            nc.scalar.mul(e2[:], Pi(3), gvc)
            nc.vector.tensor_add(out=e2[:], in0=e2[:], in1=Pi(1))
            nc.vector.tensor_mul(out=den[:], in0=e1[:], in1=e2[:])
            # rs = 1/sqrt(den): ACT sqrt + DVE reciprocal + Newton polish
            nc.scalar.sqrt(nt[:], den[:])
            nc.vector.reciprocal(rs[:], nt[:])
            for _ in range(NEWTON_ITERS):
                nc.vector.tensor_mul(out=nt[:], in0=rs[:], in1=rs[:])
                nc.vector.tensor_mul(out=nt[:], in0=nt[:], in1=den[:])
                nc.vector.tensor_scalar(nt[:], nt[:], -0.5, 1.5, AOP.mult, AOP.add)
                nc.vector.tensor_mul(out=rs[:], in0=rs[:], in1=nt[:])

            # qsum = k1*P4 + k2*P5 + k3*P6 + k4*P7
            nc.scalar.mul(qn[:], Pi(4), k1)
            nc.scalar.mul(qa[:], Pi(5), k2)
            nc.scalar.mul(qb[:], Pi(6), k3)
            nc.scalar.mul(qc[:], Pi(7), k4)
            nc.vector.tensor_add(out=qn[:], in0=qn[:], in1=qa[:])
            nc.vector.tensor_add(out=qb[:], in0=qb[:], in1=qc[:])
            nc.vector.tensor_add(out=qn[:], in0=qn[:], in1=qb[:])

            # c = qsum * rsqrt(den);   relu(c - (1-margin))
            nc.vector.tensor_mul(out=CS[:, 0:BM], in0=qn[:], in1=rs[:])
            nc.scalar.activation(CS[:, BM:2 * BM], CS[:, 0:BM], Relu,
                                 bias=margin_bias[:])

            nc.sync.dma_start(res_out[:], CS[:])

    return nc


_CACHE = {}


def _get_bass(consts):
    key = consts
    if key not in _CACHE:
        nc = _build_bass(consts)
        if not nc.is_finalized():
            nc.finalize()
        _CACHE[key] = nc
    return _CACHE[key]


def _marshal_core(A_map, P_map, i):
    """Gather mel rows + lay out time-major: [t%128, chunk, bmel-xy]."""
    xs = A_map[i * B_LOC:(i + 1) * B_LOC, 0][:, IDX_M, :].reshape(BM, T)
    ys_ = P_map[i * B_LOC:(i + 1) * B_LOC, 0][:, IDX_M, :].reshape(BM, T)
    nat = np.zeros((128, TPAD), np.float32)
    nat[0:BM, 0:T] = xs
    nat[BM:128, 0:T] = ys_
    return np.ascontiguousarray(
        nat.reshape(128, NCHUNK, 128).transpose(2, 1, 0))


def kernel(A_map, P_map, y, W1a, W2a, W1p, W2p):
    A_map = np.asarray(A_map, dtype=np.float32)
    P_map = np.asarray(P_map, dtype=np.float32)
    y = np.asarray(y)
    consts = _derived_scalars(
        np.asarray(W1a), np.asarray(W2a), np.asarray(W1p), np.asarray(W2p)
    )
    nc = _get_bass(consts)

    in_maps = [{"xy": _marshal_core(A_map, P_map, i)} for i in range(N_CORES)]
    res = run_bass_kernel_spmd(nc, in_maps, core_ids=list(range(N_CORES)))

    sumc = np.empty(B, np.float64)
    sumr = np.empty(B, np.float64)
    for i in range(N_CORES):
        cs = res.results[i]["res"].astype(np.float64)  # [128 wins, 2*BM]
        sc = cs[:, 0:BM].sum(0).reshape(B_LOC, NMEL).sum(1)
        rc = cs[:, BM:2 * BM].sum(0).reshape(B_LOC, NMEL).sum(1)
        sumc[i * B_LOC:(i + 1) * B_LOC] = sc
        sumr[i * B_LOC:(i + 1) * B_LOC] = rc

    mr = (y == 0)
    ms = (y == 1)
    nr = int(mr.sum())
    ns = int(ms.sum())
    loss_real = (nr * NWIN_TOT - sumc[mr].sum()) / (max(nr, 1) * NWIN_TOT)
    loss_spoof = sumr[ms].sum() / (max(ns, 1) * NWIN_TOT)
    stc_loss = np.float32(loss_real + SPOOF_WEIGHT * loss_spoof)
    coh = (sumc / NWIN_TOT).astype(np.float32)
    return stc_loss, coh


# revision 18
# speedup vs baseline: 1.0962x; 1.0962x over previous
"""Trainium2 Bass kernel for ASN consistency loss.

Math: the 1x1-conv heads have C_in=1, so relu(x*W1[c]) splits into
alpha/beta coefficients applied to relu(x)/relu(-x).  The per-window
cosine distances then reduce to windowed sums of 8 pointwise quantities
{x^2, y^2, xp^2, yp^2, xp*yp, xp*y, x*yp, x*y} with xp=relu(x), combined
with a handful of scalars derived from the weights.  The windowed sums
are one accumulated float32r matmul against a 0/1 window-indicator
matrix with time on the partition (contraction) axis.  The final loss
only needs per-sample sums of cos-sim and relu(cos-sim - (1-margin))
over all (mel, window) pairs, so each core emits a [128, 128] block of
per-window values; the cheap reductions finish on host.

Sharding: data-parallel over batch, 4 samples/core on 8 cores.  The 16
selected mel rows are gathered and the tile is laid out time-major on
host (pure data marshalling; all FLOPs stay on device).
"""

import numpy as np

import concourse.bass as bass
import concourse.tile as tile
from concourse import bacc, mybir
from concourse.bass import ts
from concourse.bass_utils import run_bass_kernel_spmd

# ---- constants from the nn.Module (hardcoded) ----
SR = 16000
HOP_LENGTH = 256
HOP_MS = 40
WIN_LENGTHS = (80, 160)
MARGIN = 0.2
SPOOF_WEIGHT = 0.5
MEL_SAMPLE = 16
MAX_TIME_WINDOWS = 64
SEED = 123

B, M, T = 32, 80, 2000
N_CORES = 8
B_LOC = B // N_CORES        # 4 samples per core
NMEL = MEL_SAMPLE           # 16
BM = B_LOC * NMEL           # 64 (b, mel) pairs per core
TPAD = 2048
NCHUNK = 16                 # t chunks of 128
NW = 128                    # 64 windows of w=5 + 64 of w=10
NQ = 8                      # pointwise quantities
NWIN_TOT = 2 * MAX_TIME_WINDOWS * NMEL  # 2048 window-mel pairs per sample
F32 = mybir.dt.float32
F32R = mybir.dt.float32r

NEWTON_ITERS = 1  # rsqrt refinement steps after ACT sqrt + DVE reciprocal


def _ms_to_frames(ms):
    samples = int(SR * (ms / 1000.0))
    return max(3, samples // HOP_LENGTH)


def _window_meta():
    hop = _ms_to_frames(HOP_MS)
    out = []
    for w_ms in WIN_LENGTHS:
        w = _ms_to_frames(w_ms)
        starts = np.arange(0, T - w + 1, hop, dtype=np.int64)
        if MAX_TIME_WINDOWS and starts.size > MAX_TIME_WINDOWS:
            sel = np.linspace(0, starts.size - 1, MAX_TIME_WINDOWS).astype(np.int64)
            starts = starts[sel]
        out.append((starts, w))
    return out


def _build_s():
    """Window indicator matrix, SBUF layout [128 part, NCHUNK, NW]:
    s[p, c, win] = 1.0 iff t = c*128 + p lies inside window `win`."""
    S = np.zeros((TPAD, NW), np.float32)
    col = 0
    for starts, w in _window_meta():
        for s0 in starts:
            S[s0:s0 + w, col] = 1.0
            col += 1
    assert col == NW
    return np.ascontiguousarray(S.reshape(NCHUNK, 128, NW).transpose(1, 0, 2))


IDX_M = np.sort(np.random.default_rng(SEED).permutation(M)[:MEL_SAMPLE])


def _derived_scalars(W1a, W2a, W1p, W2p):
    def alphabeta(W1, W2):
        w = W1[:, 0].astype(np.float64)
        W2 = W2.astype(np.float64)
        alpha = (W2 * (w * (w > 0))[None, :]).sum(1)
        beta = (W2 * ((-w) * (w < 0))[None, :]).sum(1)
        return alpha, beta

    aa, ba = alphabeta(W1a, W2a)
    ap, bp = alphabeta(W1p, W2p)
    caa = aa @ ap; cab = aa @ bp; cba = ba @ ap; cbb = ba @ bp
    Aa = aa @ aa; Ba = ba @ ba; Ap_ = ap @ ap; Bp_ = bp @ bp
    # q = k1 xp*yp + k2 xp*y + k3 x*yp + k4 x*y  (scaled by 1/sqrt(Ba*Bp))
    kscale = 1.0 / np.sqrt(Ba * Bp_)
    k1 = (caa + cab + cba + cbb) * kscale
    k2 = -(cab + cbb) * kscale
    k3 = -(cba + cbb) * kscale
    k4 = cbb * kscale
    # gu = Ba*(x^2 + r1*xp^2), gv = Bp*(y^2 + r2*yp^2)
    r1 = (Aa - Ba) / Ba
    r2 = (Ap_ - Bp_) / Bp_
    rA = r1 if r1 > 0 else 1.0
    return tuple(
        float(v)
        for v in (
            k1, k2, k3, k4,
            np.sqrt(rA),        # scale inside Square -> slot2 = rA*xp^2
            r1 / rA,            # gu = P0 + guc*P2   (1.0 when folded)
            r2 / rA,            # gv = P1 + gvc*P3
        )
    )


def _build_bass(consts):
    k1, k2, k3, k4, s_sq, guc, gvc = consts
    Relu = mybir.ActivationFunctionType.Relu
    Sq = mybir.ActivationFunctionType.Square
    AOP = mybir.AluOpType

    nc = bacc.Bacc()
    xy_in = nc.dram_tensor("xy", [128, NCHUNK, 128], F32, kind="ExternalInput")
    res_out = nc.dram_tensor("res", [128, 2 * BM], F32, kind="ExternalOutput")
    s_dram = nc.inline_tensor(_build_s(), name="smat")

    with tile.TileContext(nc) as tc:
        with (
            tc.tile_pool(name="main", bufs=1) as pool,
            tc.tile_pool(name="wupp", bufs=1, space="PSUM") as psum_wup,
            tc.tile_pool(name="acc", bufs=1, space="PSUM") as psum_acc,
        ):
            # Pin the ACT table set (sqrt_and_others covers Sqrt/Relu/Square)
            # with a tiny early sqrt so no mid-kernel table switch happens.
            scr = pool.tile([1, 1], F32)
            nc.scalar.sqrt(scr[:], nc.const_aps.tensor(1.0, (1, 1)))

            # PE warm-up during the input-DMA latency window: ramps the HAM
            # clock gate so the windowsum matmuls run at full rate.
            wt = pool.tile([128, 128], F32)
            nc.gpsimd.memset(wt[:], 1.0)
            wup = psum_wup.tile([128, 128], F32, tag="wup")
            for _ in range(10):
                nc.tensor.matmul(wup[:], wt[:], wt[:], is_transpose=True,
                                 start=True, stop=True, skip_group_check=True)

            margin_bias = pool.tile([128, 1], F32)
            nc.gpsimd.memset(margin_bias[:], -(1.0 - MARGIN))

            # input is pre-transposed on host: [t%128, chunk, bmel-xy]
            xyT = pool.tile([128, NCHUNK, 128], F32)
            nc.sync.dma_start(xyT[:, 0:NCHUNK // 2, :], xy_in[:, 0:NCHUNK // 2, :])
            nc.scalar.dma_start(xyT[:, NCHUNK // 2:, :], xy_in[:, NCHUNK // 2:, :])

            s_sb = pool.tile([128, NCHUNK, NW], F32R)
            nc.gpsimd.dma_start(s_sb[:].bitcast(F32), s_dram[:])

            # ---- pointwise quantities ----
            XPYP = pool.tile([128, NCHUNK, 128], F32)
            Q = pool.tile([128, NCHUNK, NQ * BM], F32R)

            # pointwise in quarters so the matmul stream can start early
            for g in range(4):
                h = ts(g, 4)
                XY = xyT[:, h, :]
                X = xyT[:, h, 0:BM]
                Y = xyT[:, h, BM:128]
                XPh = XPYP[:, h, 0:BM]
                YPh = XPYP[:, h, BM:128]

                def qs(i, h=h):
                    return Q[:, h, ts(i, BM)]

                nc.gpsimd.tensor_scalar(
                    XPYP[:, h, :], XY, 0.0, None, AOP.max
                )
                nc.vector.tensor_mul(out=qs(7), in0=X, in1=Y)          # x*y
                nc.scalar.activation(Q[:, h, 0:128], XY, Sq)           # x^2|y^2
                nc.vector.tensor_mul(out=qs(6), in0=X, in1=YPh)        # x*yp
                nc.scalar.activation(Q[:, h, 128:256], XPYP[:, h, :], Sq,
                                     scale=s_sq)                       # rA*(xp^2|yp^2)
                nc.vector.tensor_mul(out=qs(5), in0=XPh, in1=Y)        # xp*y
                nc.vector.tensor_mul(out=qs(4), in0=XPh, in1=YPh)      # xp*yp

            # ---- windowed sums: accumulate over the 16 t-chunks ----
            Pacc = psum_acc.tile([128, NQ * BM], F32, tag="pacc")
            for c in range(NCHUNK):
                nc.tensor.matmul(
                    Pacc[:],
                    s_sb[:, c, :],
                    Q[:, c, :],
                    start=(c == 0),
                    stop=(c == NCHUNK - 1),
                )

            def Pi(i):
                return Pacc[:, ts(i, BM)]

            # ---- cosine + margin terms, [128 windows, 64 bmel] ----
            # (each op touches at most one PSUM operand)
            e1 = pool.tile([128, BM], F32)
            e2 = pool.tile([128, BM], F32)
            den = pool.tile([128, BM], F32)
            rs = pool.tile([128, BM], F32)
            nt = pool.tile([128, BM], F32)
            qn = pool.tile([128, BM], F32)
            qa = pool.tile([128, BM], F32)
            qb = pool.tile([128, BM], F32)
            qc = pool.tile([128, BM], F32)
            CS = pool.tile([128, 2 * BM], F32)

            # gu = x2 + guc*xp2' ; gv = y2 + gvc*yp2'
            nc.scalar.mul(e1[:], Pi(2), guc)
            nc.vector.tensor_add(out=e1[:], in0=e1[:], in1=Pi(0))
            nc.scalar.mul(e2[:], Pi(3), gvc)
            nc.vector.tensor_add(out=e2[:], in0=e2[:], in1=Pi(1))
            nc.vector.tensor_mul(out=den[:], in0=e1[:], in1=e2[:])
            # rs = 1/sqrt(den): ACT sqrt + DVE reciprocal + Newton polish
            nc.scalar.sqrt(nt[:], den[:])
            nc.vector.reciprocal(rs[:], nt[:])
            for _ in range(NEWTON_ITERS):
                nc.vector.tensor_mul(out=nt[:], in0=rs[:], in1=rs[:])
                nc.vector.tensor_mul(out=nt[:], in0=nt[:], in1=den[:])
                nc.vector.tensor_scalar(nt[:], nt[:], -0.5, 1.5, AOP.mult, AOP.add)
                nc.vector.tensor_mul(out=rs[:], in0=rs[:], in1=nt[:])

            # qsum = k1*P4 + k2*P5 + k3*P6 + k4*P7
            nc.scalar.mul(qn[:], Pi(4), k1)
            nc.scalar.mul(qa[:], Pi(5), k2)
            nc.scalar.mul(qb[:], Pi(6), k3)
            nc.scalar.mul(qc[:], Pi(7), k4)
            nc.vector.tensor_add(out=qn[:], in0=qn[:], in1=qa[:])
            nc.vector.tensor_add(out=qb[:], in0=qb[:], in1=qc[:])
            nc.vector.tensor_add(out=qn[:], in0=qn[:], in1=qb[:])

            # c = qsum * rsqrt(den);   relu(c - (1-margin))
            nc.vector.tensor_mul(out=CS[:, 0:BM], in0=qn[:], in1=rs[:])
            nc.scalar.activation(CS[:, BM:2 * BM], CS[:, 0:BM], Relu,
                                 bias=margin_bias[:])

            nc.sync.dma_start(res_out[:], CS[:])

    return nc


_CACHE = {}


def _get_bass(consts):
    key = consts
    if key not in _CACHE:
        nc = _build_bass(consts)
        if not nc.is_finalized():
            nc.finalize()
        _CACHE[key] = nc
    return _CACHE[key]


def _marshal_core(A_map, P_map, i):
    """Gather mel rows + lay out time-major: [t%128, chunk, bmel-xy]."""
    xs = A_map[i * B_LOC:(i + 1) * B_LOC, 0][:, IDX_M, :].reshape(BM, T)
    ys_ = P_map[i * B_LOC:(i + 1) * B_LOC, 0][:, IDX_M, :].reshape(BM, T)
    nat = np.zeros((128, TPAD), np.float32)
    nat[0:BM, 0:T] = xs
    nat[BM:128, 0:T] = ys_
    return np.ascontiguousarray(
        nat.reshape(128, NCHUNK, 128).transpose(2, 1, 0))


def kernel(A_map, P_map, y, W1a, W2a, W1p, W2p):
    A_map = np.asarray(A_map, dtype=np.float32)
    P_map = np.asarray(P_map, dtype=np.float32)
    y = np.asarray(y)
    consts = _derived_scalars(
        np.asarray(W1a), np.asarray(W2a), np.asarray(W1p), np.asarray(W2p)
    )
    nc = _get_bass(consts)

    in_maps = [{"xy": _marshal_core(A_map, P_map, i)} for i in range(N_CORES)]
    res = run_bass_kernel_spmd(nc, in_maps, core_ids=list(range(N_CORES)))

    sumc = np.empty(B, np.float64)
    sumr = np.empty(B, np.float64)
    for i in range(N_CORES):
        cs = res.results[i]["res"].astype(np.float64)  # [128 wins, 2*BM]
        sc = cs[:, 0:BM].sum(0).reshape(B_LOC, NMEL).sum(1)
        rc = cs[:, BM:2 * BM].sum(0).reshape(B_LOC, NMEL).sum(1)
        sumc[i * B_LOC:(i + 1) * B_LOC] = sc
        sumr[i * B_LOC:(i + 1) * B_LOC] = rc

    mr = (y == 0)
    ms = (y == 1)
    nr = int(mr.sum())
    ns = int(ms.sum())
    loss_real = (nr * NWIN_TOT - sumc[mr].sum()) / (max(nr, 1) * NWIN_TOT)
    loss_spoof = sumr[ms].sum() / (max(ns, 1) * NWIN_TOT)
    stc_loss = np.float32(loss_real + SPOOF_WEIGHT * loss_spoof)
    coh = (sumc / NWIN_TOT).astype(np.float32)
    return stc_loss, coh


# revision 31
# speedup vs baseline: 1.4815x; 1.3515x over previous
"""Trainium2 Bass kernel for ASN consistency loss.

Math: the 1x1-conv heads have C_in=1, so relu(x*W1[c]) splits into
alpha/beta coefficients applied to relu(x)/relu(-x).  The per-window
cosine distances then reduce to windowed sums of 8 pointwise quantities
{x^2, y^2, xp^2, yp^2, xp*yp, xp*y, x*yp, x*y} with xp=relu(x), combined
with a handful of scalars derived from the weights.  The windowed sums
are one accumulated float32r matmul against a 0/1 window-indicator
matrix with time on the partition (contraction) axis.  The final loss
only needs per-sample sums of cos-sim and relu(cos-sim - (1-margin))
over all (mel, window) pairs, so each core emits a [128, 128] block of
per-window values; the cheap reductions finish on host.

Sharding: data-parallel over batch, 4 samples/core on 8 cores.  The 16
selected mel rows are gathered and the tile is laid out time-major on
host (pure data marshalling; all FLOPs stay on device).
"""

import numpy as np

import concourse.bass as bass
import concourse.tile as tile
from concourse import bacc, mybir
from concourse.bass import ts
from concourse.bass_utils import run_bass_kernel_spmd

# ---- constants from the nn.Module (hardcoded) ----
SR = 16000
HOP_LENGTH = 256
HOP_MS = 40
WIN_LENGTHS = (80, 160)
MARGIN = 0.2
SPOOF_WEIGHT = 0.5
MEL_SAMPLE = 16
MAX_TIME_WINDOWS = 64
SEED = 123

B, M, T = 32, 80, 2000
N_CORES = 8
B_LOC = B // N_CORES        # 4 samples per core
NMEL = MEL_SAMPLE           # 16
BM = B_LOC * NMEL           # 64 (b, mel) pairs per core
TPAD = 2048
NCHUNK = 16                 # t chunks of 128
NW = 128                    # 64 windows of w=5 + 64 of w=10
NQ = 8                      # pointwise quantities
NWIN_TOT = 2 * MAX_TIME_WINDOWS * NMEL  # 2048 window-mel pairs per sample
F32 = mybir.dt.float32
F32R = mybir.dt.float32r

# rsqrt refinement steps after ACT sqrt + DVE reciprocal; HW-measured:
# 0 vs 1 iteration both land at ~1e-5 rel err (f32r rounding dominates)
NEWTON_ITERS = 0


def _ms_to_frames(ms):
    samples = int(SR * (ms / 1000.0))
    return max(3, samples // HOP_LENGTH)


def _window_meta():
    hop = _ms_to_frames(HOP_MS)
    out = []
    for w_ms in WIN_LENGTHS:
        w = _ms_to_frames(w_ms)
        starts = np.arange(0, T - w + 1, hop, dtype=np.int64)
        if MAX_TIME_WINDOWS and starts.size > MAX_TIME_WINDOWS:
            sel = np.linspace(0, starts.size - 1, MAX_TIME_WINDOWS).astype(np.int64)
            starts = starts[sel]
        out.append((starts, w))
    return out


def _build_s():
    """Window indicator matrix, SBUF layout [128 part, NCHUNK, NW]:
    s[p, c, win] = 1.0 iff t = c*128 + p lies inside window `win`."""
    S = np.zeros((TPAD, NW), np.float32)
    col = 0
    for starts, w in _window_meta():
        for s0 in starts:
            S[s0:s0 + w, col] = 1.0
            col += 1
    assert col == NW
    return np.ascontiguousarray(S.reshape(NCHUNK, 128, NW).transpose(1, 0, 2))


IDX_M = np.sort(np.random.default_rng(SEED).permutation(M)[:MEL_SAMPLE])


def _derived_scalars(W1a, W2a, W1p, W2p):
    def alphabeta(W1, W2):
        w = W1[:, 0].astype(np.float64)
        W2 = W2.astype(np.float64)
        alpha = (W2 * (w * (w > 0))[None, :]).sum(1)
        beta = (W2 * ((-w) * (w < 0))[None, :]).sum(1)
        return alpha, beta

    aa, ba = alphabeta(W1a, W2a)
    ap, bp = alphabeta(W1p, W2p)
    caa = aa @ ap; cab = aa @ bp; cba = ba @ ap; cbb = ba @ bp
    Aa = aa @ aa; Ba = ba @ ba; Ap_ = ap @ ap; Bp_ = bp @ bp
    # q = k1 xp*yp + k2 xp*y + k3 x*yp + k4 x*y  (scaled by 1/sqrt(Ba*Bp))
    kscale = 1.0 / np.sqrt(Ba * Bp_)
    k1 = (caa + cab + cba + cbb) * kscale
    k2 = -(cab + cbb) * kscale
    k3 = -(cba + cbb) * kscale
    k4 = cbb * kscale
    # gu = Ba*(x^2 + r1*xp^2), gv = Bp*(y^2 + r2*yp^2)
    r1 = (Aa - Ba) / Ba
    r2 = (Ap_ - Bp_) / Bp_
    rA = r1 if r1 > 0 else 1.0
    return tuple(
        float(v)
        for v in (
            k1, k2, k3, k4,
            np.sqrt(rA),        # scale inside Square -> slot2 = rA*xp^2
            r1 / rA,            # gu = P0 + guc*P2   (1.0 when folded)
            r2 / rA,            # gv = P1 + gvc*P3
        )
    )


def _build_bass(consts):
    k1, k2, k3, k4, s_sq, guc, gvc = consts
    Relu = mybir.ActivationFunctionType.Relu
    Sq = mybir.ActivationFunctionType.Square
    AOP = mybir.AluOpType

    nc = bacc.Bacc()
    xy_in = nc.dram_tensor("xy", [128, NCHUNK, 128], F32, kind="ExternalInput")
    res_out = nc.dram_tensor("res", [128, 2 * BM], F32, kind="ExternalOutput")
    s_dram = nc.inline_tensor(_build_s(), name="smat")

    with tile.TileContext(nc) as tc:
        with (
            tc.tile_pool(name="main", bufs=1) as pool,
            tc.tile_pool(name="wupp", bufs=1, space="PSUM") as psum_wup,
            tc.tile_pool(name="acc", bufs=1, space="PSUM") as psum_acc,
        ):
            # Pin the ACT table set (sqrt_and_others covers Sqrt/Relu/Square)
            # with a tiny early sqrt so no mid-kernel table switch happens.
            scr = pool.tile([1, 1], F32)
            nc.scalar.sqrt(scr[:], nc.const_aps.tensor(1.0, (1, 1)))

            # PE warm-up during the input-DMA latency window: ramps the HAM
            # clock gate so the windowsum matmuls run at full rate.
            wt = pool.tile([128, 128], F32)
            nc.gpsimd.memset(wt[:], 1.0)
            wup = psum_wup.tile([128, 128], F32, tag="wup")
            for _ in range(24):
                nc.tensor.matmul(wup[:], wt[:], wt[:], is_transpose=True,
                                 start=True, stop=True, skip_group_check=True)

            # input is pre-transposed on host: [t%128, chunk, bmel-xy]
            xyT = pool.tile([128, NCHUNK, 128], F32)
            for g in range(4):
                nc.sync.dma_start(xyT[:, ts(g, 4), :], xy_in[:, ts(g, 4), :])

            # S rides the same FIFO ring behind the input pieces so its
            # transfer never delays the input semaphores.
            s_sb = pool.tile([128, NCHUNK, NW], F32R)
            for g in range(4):
                nc.sync.dma_start(s_sb[:, ts(g, 4), :].bitcast(F32),
                                  s_dram[:, ts(g, 4), :])

            # ---- pointwise quantities ----
            XPYP = pool.tile([128, NCHUNK, 128], F32)
            Q = pool.tile([128, NCHUNK, NQ * BM], F32R)

            # pointwise in quarters so the matmul stream can start early;
            # work spread over Pool (relu + 2 products), DVE (products +
            # one square pair) and ACT (squares)
            for g in range(4):
                h = ts(g, 4)
                XY = xyT[:, h, :]
                X = xyT[:, h, 0:BM]
                Y = xyT[:, h, BM:128]
                XPh = XPYP[:, h, 0:BM]
                YPh = XPYP[:, h, BM:128]

                def qs(i, h=h):
                    return Q[:, h, ts(i, BM)]

                nc.gpsimd.tensor_scalar(
                    XPYP[:, h, :], XY, 0.0, None, AOP.max
                )
                nc.vector.tensor_mul(out=qs(7), in0=X, in1=Y)          # x*y
                if g == 3:
                    nc.vector.tensor_tensor(
                        Q[:, h, 0:128], XY, XY, AOP.mult)              # x^2|y^2
                else:
                    nc.scalar.activation(Q[:, h, 0:128], XY, Sq)       # x^2|y^2
                nc.vector.tensor_mul(out=qs(6), in0=X, in1=YPh)        # x*yp
                nc.scalar.activation(Q[:, h, 128:256], XPYP[:, h, :], Sq,
                                     scale=s_sq)                       # rA*(xp^2|yp^2)
                if g == 3:
                    nc.gpsimd.tensor_tensor(qs(5), XPh, Y, AOP.mult)   # xp*y
                    nc.gpsimd.tensor_tensor(qs(4), XPh, YPh, AOP.mult)  # xp*yp
                else:
                    nc.vector.tensor_mul(out=qs(5), in0=XPh, in1=Y)    # xp*y
                    nc.vector.tensor_mul(out=qs(4), in0=XPh, in1=YPh)  # xp*yp

            # ---- windowed sums: accumulate over the 16 t-chunks ----
            # two PSUM banks: A = norm slots 0-3 (DVE side), B = product
            # slots 4-7 (ACT side) so the post phase runs bank-parallel
            # bank A (squares, ready first) fully before bank B (products)
            # so the norm branch of the epilogue overlaps B's matmul tail
            PaccA = psum_acc.tile([128, 4 * BM], F32, tag="pacc_a")
            PaccB = psum_acc.tile([128, 4 * BM], F32, tag="pacc_b")
            for c in range(NCHUNK):
                nc.tensor.matmul(
                    PaccA[:], s_sb[:, c, :], Q[:, c, 0:4 * BM],
                    start=(c == 0), stop=(c == NCHUNK - 1),
                )
            for c in range(NCHUNK):
                nc.tensor.matmul(
                    PaccB[:], s_sb[:, c, :], Q[:, c, 4 * BM:8 * BM],
                    start=(c == 0), stop=(c == NCHUNK - 1),
                )

            def Pi(i):
                return (PaccA if i < 4 else PaccB)[:, ts(i % 4, BM)]

            # ---- cosine + margin terms, [128 windows, 64 bmel] ----
            # (each op touches at most one PSUM operand)
            e1 = pool.tile([128, BM], F32)
            e2 = pool.tile([128, BM], F32)
            den = pool.tile([128, BM], F32)
            rs = pool.tile([128, BM], F32)
            nt = pool.tile([128, BM], F32)
            qn = pool.tile([128, BM], F32)
            qa = pool.tile([128, BM], F32)
            qb = pool.tile([128, BM], F32)
            qc = pool.tile([128, BM], F32)
            CS = pool.tile([128, 2 * BM], F32)

            # den branch on DVE; qsum muls on ACT (bank B) in parallel;
            # qsum adds slotted into the DVE gaps of the den chain.
            nc.vector.tensor_scalar_mul(e1[:], Pi(2), guc)
            nc.scalar.mul(qn[:], Pi(4), k1)
            nc.vector.tensor_add(out=e1[:], in0=e1[:], in1=Pi(0))
            nc.scalar.mul(qa[:], Pi(5), k2)
            nc.vector.tensor_scalar_mul(e2[:], Pi(3), gvc)
            nc.scalar.mul(qb[:], Pi(6), k3)
            nc.vector.tensor_add(out=e2[:], in0=e2[:], in1=Pi(1))
            nc.scalar.mul(qc[:], Pi(7), k4)
            nc.vector.tensor_mul(out=den[:], in0=e1[:], in1=e2[:])
            # rs = 1/sqrt(den): ACT sqrt + exact DVE reciprocal
            nc.scalar.sqrt(nt[:], den[:])
            nc.vector.tensor_add(out=qn[:], in0=qn[:], in1=qa[:])
            nc.vector.tensor_add(out=qb[:], in0=qb[:], in1=qc[:])
            nc.vector.reciprocal(rs[:], nt[:])
            nc.vector.tensor_add(out=qn[:], in0=qn[:], in1=qb[:])
            for _ in range(NEWTON_ITERS):
                nc.vector.tensor_mul(out=nt[:], in0=rs[:], in1=rs[:])
                nc.vector.tensor_mul(out=nt[:], in0=nt[:], in1=den[:])
                nc.vector.tensor_scalar(nt[:], nt[:], -0.5, 1.5, AOP.mult, AOP.add)
                nc.vector.tensor_mul(out=rs[:], in0=rs[:], in1=nt[:])

            # c = qsum * rsqrt(den);   relu(c - (1-margin)) fused on DVE
            nc.vector.tensor_mul(out=CS[:, 0:BM], in0=qn[:], in1=rs[:])
            nc.vector.tensor_scalar(CS[:, BM:2 * BM], CS[:, 0:BM],
                                    -(1.0 - MARGIN), 0.0, AOP.add, AOP.max)

            nc.sync.dma_start(res_out[:], CS[:])

    return nc


_CACHE = {}


def _get_bass(consts):
    key = consts
    if key not in _CACHE:
        nc = _build_bass(consts)
        if not nc.is_finalized():
            nc.finalize()
        _CACHE[key] = nc
    return _CACHE[key]


def _marshal_core(A_map, P_map, i):
    """Gather mel rows + lay out time-major: [t%128, chunk, bmel-xy]."""
    xs = A_map[i * B_LOC:(i + 1) * B_LOC, 0][:, IDX_M, :].reshape(BM, T)
    ys_ = P_map[i * B_LOC:(i + 1) * B_LOC, 0][:, IDX_M, :].reshape(BM, T)
    nat = np.zeros((128, TPAD), np.float32)
    nat[0:BM, 0:T] = xs
    nat[BM:128, 0:T] = ys_
    return np.ascontiguousarray(
        nat.reshape(128, NCHUNK, 128).transpose(2, 1, 0))


def kernel(A_map, P_map, y, W1a, W2a, W1p, W2p):
    A_map = np.asarray(A_map, dtype=np.float32)
    P_map = np.asarray(P_map, dtype=np.float32)
    y = np.asarray(y)
    consts = _derived_scalars(
        np.asarray(W1a), np.asarray(W2a), np.asarray(W1p), np.asarray(W2p)
    )
    nc = _get_bass(consts)

    in_maps = [{"xy": _marshal_core(A_map, P_map, i)} for i in range(N_CORES)]
    res = run_bass_kernel_spmd(nc, in_maps, core_ids=list(range(N_CORES)))

    sumc = np.empty(B, np.float64)
    sumr = np.empty(B, np.float64)
    for i in range(N_CORES):
        cs = res.results[i]["res"].astype(np.float64)  # [128 wins, 2*BM]
        sc = cs[:, 0:BM].sum(0).reshape(B_LOC, NMEL).sum(1)
        rc = cs[:, BM:2 * BM].sum(0).reshape(B_LOC, NMEL).sum(1)
        sumc[i * B_LOC:(i + 1) * B_LOC] = sc
        sumr[i * B_LOC:(i + 1) * B_LOC] = rc

    mr = (y == 0)
    ms = (y == 1)
    nr = int(mr.sum())
    ns = int(ms.sum())
    loss_real = (nr * NWIN_TOT - sumc[mr].sum()) / (max(nr, 1) * NWIN_TOT)
    loss_spoof = sumr[ms].sum() / (max(ns, 1) * NWIN_TOT)
    stc_loss = np.float32(loss_real + SPOOF_WEIGHT * loss_spoof)
    coh = (sumc / NWIN_TOT).astype(np.float32)
    return stc_loss, coh
